# revision 47
# baseline (speedup 1.0000x reference)
"""Trainium2 Bass kernel for nn_LorentzGNN (2x GATv2 + Lorentz head), 8-core SPMD.

Sharding: nodes (and their in-edges) are partitioned contiguously across 8 cores
(2048 nodes each). Layer-1 source transform (xl1) is computed replicated on
every core from the full x table (cheaper than an AllGather under the TRN2
collective cost model); layer-2's xl table is AllGathered once. Per-edge work
uses dma_gather (edge-major + feat-major transposed gathers) and host-built
0/1 segment matrices fed to the PE as matmuls (segment softmax + aggregation).
Leaky-relu is a single Prelu activation pass; per-jb exponentials batch into
one Exp per dst tile; gelu runs once per layer on the transposed h1 buffer.
Graph-level ops (centroid + Lorentz MLP over the 16 graphs each core owns)
are data-parallel; host concatenates.
"""
import numpy as np
import ml_dtypes

# ---------- problem constants (hardcoded per contract) ----------
N, E, B = 16384, 131072, 128
FT, HEADS, C = 512, 4, 128
NCORES = 8
SHARD = N // NCORES            # 2048
P = 128
NT = SHARD // P                # 16 dst tiles per core
NTF = N // P                   # 128 tiles in the full node table
GPC = B // NCORES              # 16 graphs per core
LEAK = 0.2

_cache = {}


# ============================ host-side prep ============================

def _prep_edges(edge_index):
    src = np.concatenate([edge_index[0], np.arange(N)]).astype(np.int64)
    dst = np.concatenate([edge_index[1], np.arange(N)]).astype(np.int64)
    order = np.argsort(dst, kind="stable")
    src, dst = src[order], dst[order]
    ntiles = N // P
    counts = np.bincount(dst // P, minlength=ntiles)
    LP = int(np.ceil(max(counts.max(), 1) / 128) * 128)
    NJ = LP // P
    srcs = np.zeros((ntiles, LP), np.int16)
    dloc = np.full((ntiles, LP), -1, np.int32)
    starts = np.concatenate([[0], np.cumsum(counts)])
    for t in range(ntiles):
        c = counts[t]
        srcs[t, :c] = src[starts[t]:starts[t] + c]
        dloc[t, :c] = dst[starts[t]:starts[t] + c] - t * P
    # segment matrices
    sdt = np.zeros((ntiles, P, LP), np.float16)          # [dst, j]
    jj = np.arange(LP)
    for t in range(ntiles):
        v = dloc[t] >= 0
        sdt[t, dloc[t, v], jj[v]] = 1.0
    sjt = np.ascontiguousarray(sdt.transpose(0, 2, 1))   # [j, dst] edge-major
    # idx buffers wrapped in 16 partitions, replicated to 128
    idx = srcs.reshape(ntiles, LP // 16, 16).transpose(0, 2, 1)  # [t, 16, LP/16]
    idx = np.tile(idx, (1, 8, 1)).astype(np.int16)               # [t, 128, LP/16]
    return srcs, sdt, sjt, idx, LP, NJ


def _interleave_k(w, kchunks):
    """[K*128, N] -> [128, K, N] -> host layout [128, K*N] for SBUF."""
    K, Nn = w.shape
    assert K == kchunks * 128
    return np.ascontiguousarray(w.reshape(kchunks, 128, Nn).transpose(1, 0, 2))


def _aug_w(W, b, kpad, npad=None):
    """stack rows [W; b; 0-pad] to kpad rows, optionally pad cols to npad."""
    K, Nn = W.shape
    out = np.zeros((kpad, Nn if npad is None else npad), np.float32)
    out[:K, :Nn] = W
    out[K, :Nn] = b
    return out


# ============================ kernel build ============================

def _build(LP, NJ, ex_dtype_name):
    import concourse.bass as bass  # noqa
    import concourse.bacc as bacc
    import concourse.tile as tile
    from concourse import mybir
    from concourse.library_config import mlp as gpsimd_mlp

    f32, f16 = mybir.dt.float32, mybir.dt.float16
    bf16, i16 = mybir.dt.bfloat16, mybir.dt.int16
    f8 = mybir.dt.float8e4
    EXD = {"float16": f16, "bfloat16": bf16}[ex_dtype_name]
    AF = mybir.ActivationFunctionType
    ALU = mybir.AluOpType
    SC = [(o, min(512, LP - o)) for o in range(0, LP, 512)]  # lr psum slices

    nc = bacc.Bacc("TRN2", target_bir_lowering=False, debug=False,
                   num_devices=NCORES)
    groups = [list(range(NCORES))]

    # ---- DRAM I/O (per-core, same program) ----
    xTf = nc.dram_tensor("xTf", [128, 4 * N], f16, kind="ExternalInput")
    xTl = nc.dram_tensor("xTl", [128, 5 * SHARD], f16, kind="ExternalInput")
    w1l = nc.dram_tensor("w1l", [128, 4 * FT], f16, kind="ExternalInput")
    w1r = nc.dram_tensor("w1r", [128, 5 * FT], f16, kind="ExternalInput")
    w2l = nc.dram_tensor("w2l", [128, 4 * FT], f16, kind="ExternalInput")
    w2r = nc.dram_tensor("w2r", [128, 4 * FT], f16, kind="ExternalInput")
    b2r_d = nc.dram_tensor("b2rows", [1, FT], f16, kind="ExternalInput")
    a1_d = nc.dram_tensor("a1", [128, 4 * HEADS], f16, kind="ExternalInput")
    a2_d = nc.dram_tensor("a2", [128, 4 * HEADS], f16, kind="ExternalInput")
    b1f_d = nc.dram_tensor("b1full", [128, FT], f32, kind="ExternalInput")
    b2f_d = nc.dram_tensor("b2full", [128, FT], f32, kind="ExternalInput")
    sdt_d = nc.dram_tensor("sdt", [NT, 128, LP], f8, kind="ExternalInput")
    sj_d = nc.dram_tensor("sj", [NT, 128, NJ * 128], f8, kind="ExternalInput")
    idx_d = nc.dram_tensor("idx", [NT, 128, LP // 16], i16, kind="ExternalInput")
    ecols_d = nc.dram_tensor("ecols", [128, NT * GPC], f16, kind="ExternalInput")
    ident_d = nc.dram_tensor("ident", [128, 128], f16, kind="ExternalInput")
    wa_d = nc.dram_tensor("wa", [128, 5 * 2176], f16, kind="ExternalInput")
    wb_d = nc.dram_tensor("wb", [128, 17 * 640], f16, kind="ExternalInput")
    wf_d = nc.dram_tensor("wf", [128, 5 * 640], f16, kind="ExternalInput")
    sabf_d = nc.dram_tensor("sabf", [16, 3], f32, kind="ExternalInput")  # sa, sb, sf

    xl1_tb = nc.dram_tensor("xl1_tb", [N, FT], f16)
    xl2_sh = nc.dram_tensor("xl2_sh", [SHARD, FT], f16)
    xl2_tb = nc.dram_tensor("xl2_tb", [N, FT], f16, addr_space="Shared")
    zout = nc.dram_tensor("zout", [GPC, FT + 1], f32, kind="ExternalOutput")
    gmout = nc.dram_tensor("gmout", [GPC, FT + 1], f32, kind="ExternalOutput")

    with tile.TileContext(nc, num_cores=NCORES) as tc:
        import contextlib
        est = contextlib.ExitStack()
        nc.gpsimd.load_library(gpsimd_mlp)
        nregs = {n: nc.gpsimd.to_reg(n)
                 for n in sorted({n for _, n in SC} | {LP})}
        # persistent pool: survives into the MLP phase
        pers = est.enter_context(tc.tile_pool(name="pers", bufs=1))
        with contextlib.ExitStack() as est2:
            cpool = est2.enter_context(tc.tile_pool(name="consts", bufs=1))
            xsp = est2.enter_context(tc.tile_pool(name="xs", bufs=4))
            sbp = est2.enter_context(tc.tile_pool(name="stream", bufs=2))
            smp = est2.enter_context(tc.tile_pool(name="small", bufs=2))
            psb = est2.enter_context(tc.tile_pool(name="psb", bufs=4, space="PSUM"))
            pss = est2.enter_context(tc.tile_pool(name="pss", bufs=2, space="PSUM"))

            # ---- consts (only w1l blocks the first matmuls; the rest are
            # issued after transform1 in program order so the SP queue drains
            # them behind the x-stream) ----
            w1l_s = cpool.tile([128, 4 * FT], f16, name="w1l_s")
            nc.sync.dma_start(w1l_s[:], w1l[:])
            w1r_s = cpool.tile([128, 5 * FT], f16, name="w1r_s")
            nc.sync.dma_start(w1r_s[:], w1r[:])
            w2l_s = cpool.tile([128, 4 * FT], f16, name="w2l_s")
            w2r_s = cpool.tile([128, 4 * FT], f16, name="w2r_s")
            b2ra_s = cpool.tile([1, FT], f16, name="b2ra_s")
            a1_s = cpool.tile([128, 4 * HEADS], f16, name="a1_s")
            a2_s = cpool.tile([128, 4 * HEADS], f16, name="a2_s")
            b1f_s = cpool.tile([128, FT], f32, name="b1f_s")
            b2f_s = cpool.tile([128, FT], f32, name="b2f_s")
            ident_s = pers.tile([128, 128], f16, name="ident_s")
            ecols_s = pers.tile([128, NT * GPC], f16, name="ecols_s")
            ones1 = cpool.tile([1, FT], f16, name="ones1")

            def load_consts():
                nc.sync.dma_start(a1_s[:], a1_d[:])
                nc.sync.dma_start(ident_s[:], ident_d[:])
                nc.sync.dma_start(b1f_s[:], b1f_d[:])
                nc.sync.dma_start(w2l_s[:], w2l[:])
                nc.sync.dma_start(w2r_s[:], w2r[:])
                nc.sync.dma_start(b2ra_s[:], b2r_d[0:1, :])
                nc.sync.dma_start(a2_s[:], a2_d[:])
                nc.sync.dma_start(b2f_s[:], b2f_d[:])
                nc.sync.dma_start(ecols_s[:], ecols_d[:])
                nc.vector.memset(ones1[:], 1.0)

            # residents
            xr_s = cpool.tile([128, NT * FT], f16, name="xr_s")
            h1T_s = cpool.tile([128, NT * FT], f16, name="h1T_s")
            h1T_g = h1T_s[:].rearrange("p (t k n) -> p t k n", t=NT, k=4)
            h2_s = pers.tile([128, NT * FT], f16, name="h2_s")
            acc16 = pers.tile([128, NT], f32, name="acc16")
            tal = pers.tile([128, NT], f32, name="tal")
            tal16 = pers.tile([128, NT], f16, name="tal16")
            z0p = pers.tile([16, 640], f16, name="z0p")

            def sqrt_nr(out_ap, x_ap, pool, pfx, x_plus=None):
                """out = sqrt(x [+ x_plus]) via the Sqrt LUT (~4e-3 rel)."""
                bias = 0.0 if x_plus is None else float(x_plus)
                nc.scalar.activation(out_ap, x_ap, AF.Sqrt, bias=bias)

            # ---------------- transform 1: full xl1 table + local xr ----
            TB = 4     # node tiles per input DMA
            WB = 4     # node tiles per xl1_tb output DMA
            def transform1():
                xTf_v = xTf[:].rearrange("p (k n) -> p k n", k=4)
                w1l_v = w1l_s[:].rearrange("p (k n) -> p k n", k=4)
                w1r_v = w1r_s[:].rearrange("p (k n) -> p k n", k=5)
                xlt = None
                for t in range(NTF):
                    if t % TB == 0:
                        xt = xsp.tile([128, 4 * TB * 128], f16, tag="xt",
                                      name="xt")
                        nc.sync.dma_start(
                            xt[:].rearrange("p (k n) -> p k n", k=4),
                            xTf_v[:, :, t * 128:(t + TB) * 128])
                        xt_v = xt[:].rearrange("p (k n) -> p k n", k=4)
                    ti = t % TB
                    pl = psb.tile([128, FT], f32, tag="pbig", name="pl")
                    for kc in range(4):
                        nc.tensor.matmul(
                            pl[:], lhsT=xt_v[:, kc, ti * 128:(ti + 1) * 128],
                            rhs=w1l_v[:, kc, :],
                            start=(kc == 0), stop=(kc == 3))
                    if t % WB == 0:
                        xlt = smp.tile([128, WB * FT], f16, tag="xlt",
                                       name="xlt", bufs=3)
                    wi = t % WB
                    if t % 2 == 0:
                        nc.scalar.activation(xlt[:, wi * FT:(wi + 1) * FT], pl[:],
                                             AF.Copy)
                    else:
                        nc.vector.tensor_copy(xlt[:, wi * FT:(wi + 1) * FT],
                                              pl[:])
                    if wi == WB - 1:
                        nc.sync.dma_start(
                            xl1_tb[(t - WB + 1) * 128:(t + 1) * 128, :]
                            .rearrange("(t p) n -> p t n", t=WB),
                            xlt[:].rearrange("p (t n) -> p t n", t=WB))
                # local xr1 from the local shard slice of xT
                xTl_v = xTl[:].rearrange("p (k n) -> p k n", k=5)
                for t in range(NT):
                    if t % TB == 0:
                        xt = xsp.tile([128, 5 * TB * 128], f16, tag="xt",
                                      name="xtl")
                        nc.sync.dma_start(
                            xt[:].rearrange("p (k n) -> p k n", k=5),
                            xTl_v[:, :, t * 128:(t + TB) * 128])
                        xt_v = xt[:].rearrange("p (k n) -> p k n", k=5)
                    ti = t % TB
                    pr = psb.tile([128, FT], f32, tag="pbig", name="pr")
                    for kc in range(5):
                        nc.tensor.matmul(
                            pr[:], lhsT=xt_v[:, kc, ti * 128:(ti + 1) * 128],
                            rhs=w1r_v[:, kc, :],
                            start=(kc == 0), stop=(kc == 4))
                    nc.vector.tensor_copy(xr_s[:, t * FT:(t + 1) * FT], pr[:])

            # ---------------- transform 2: local xl2/xr2 from h1 ----------
            def t2_xl():
                w2l_v = w2l_s[:].rearrange("p (k n) -> p k n", k=4)
                for t in range(NT):
                    pl = psb.tile([128, FT], f32, tag="pbig", name="pl2")
                    for kc in range(4):
                        nc.tensor.matmul(pl[:], lhsT=h1T_g[:, t, kc, :],
                                         rhs=w2l_v[:, kc, :],
                                         start=(kc == 0), stop=(kc == 3))
                    xlt = smp.tile([128, FT], f16, tag="xlt", name="xlt2", bufs=3)
                    nc.scalar.activation(xlt[:], pl[:], AF.Copy)
                    nc.sync.dma_start(xl2_sh[t * 128:(t + 1) * 128, :], xlt[:])

            def t2_xr():
                w2r_v = w2r_s[:].rearrange("p (k n) -> p k n", k=4)
                for t in range(NT):
                    pr = psb.tile([128, FT], f32, tag="pbig", name="pr2")
                    for kc in range(4):
                        nc.tensor.matmul(pr[:], lhsT=h1T_g[:, t, kc, :],
                                         rhs=w2r_v[:, kc, :],
                                         start=(kc == 0), stop=False)
                    nc.tensor.matmul(pr[:], lhsT=ones1[:, 0:128], rhs=b2ra_s[:],
                                     start=False, stop=True)
                    nc.vector.tensor_copy(xr_s[:, t * FT:(t + 1) * FT], pr[:])

            # ---------------- GATv2 edge phase ----------------
            def edge_layer(layer, table, a_s):
                """message passing; writes h1_s (layer1) or h2_s+acc16 (layer2)."""
                for t in range(NT):
                    idxt = smp.tile([128, LP // 16], i16, tag="idxt", name="idxt", bufs=3)
                    nc.sync.dma_start(idxt[:], idx_d[t, :, :])
                    sdtt = sbp.tile([128, LP], f8, tag="sdtt", name="sdtt")
                    nc.sync.dma_start(sdtt[:], sdt_d[t, :, :])
                    sjt = sbp.tile([128, NJ * 128], f8, tag="sjt", name="sjt")
                    nc.sync.dma_start(sjt[:], sj_d[t, :, :])
                    sj_v = sjt[:].rearrange("p (j d) -> p j d", j=NJ)

                    # xlgT layout: per SC-block contiguous [4, n] (block at 4*o)
                    xlgT = sbp.tile([128, 4 * LP], f16, tag="xlgT", name="xlgT")
                    xlg = sbp.tile([128, NJ * FT], f16, tag="xlg", name="xlg")
                    xlg_w = xlg[:].rearrange("p (j n) -> p j n", j=NJ)
                    for (o, n) in SC:
                        nc.gpsimd.dma_gather(
                            xlgT[:, 4 * o:4 * (o + n)].rearrange(
                                "p (c j) -> p c j", c=4), table[:],
                            idxt[:, o // 16:(o + n) // 16], n, nregs[n], FT,
                            transpose=True)
                        nc.gpsimd.dma_gather(
                            xlg_w[:, o // 128:(o + n) // 128, :], table[:],
                            idxt[:, o // 16:(o + n) // 16], n, nregs[n], FT)
                    xlg_v = xlg[:].rearrange("p (j n) -> p j n", j=NJ)

                    # lr = prelu(xl[src] + xr[dst]) in [feat, edge] layout
                    xr_t = xr_s[:].rearrange("p (t n) -> p t n", t=NT)[:, t, :]
                    lr = sbp.tile([128, 4 * LP], f16, tag="lr", name="lr", bufs=1)
                    lr_v = lr[:].rearrange("p (c j) -> p c j", c=4)
                    for fc in range(4):
                        for (o, n) in SC:
                            ps = psb.tile([128, 512], f32, tag="pbig", name="ps")
                            nc.tensor.matmul(ps[:, :n],
                                             lhsT=xr_t[:, fc * 128:(fc + 1) * 128],
                                             rhs=sdtt[:, o:o + n],
                                             start=True, stop=False)
                            nc.tensor.matmul(ps[:, :n], lhsT=ident_s[:],
                                             rhs=xlgT[:, 4 * o + fc * n:
                                                      4 * o + (fc + 1) * n],
                                             start=False, stop=True)
                            nc.scalar.activation(lr_v[:, fc, o:o + n], ps[:, :n],
                                                 AF.Prelu, alpha=LEAK)

                    # attention logits for all NJ blocks into one psum [128, 4*NJ]
                    a_v = a_s[:].rearrange("p (c h) -> p c h", c=4)
                    pe = pss.tile([128, HEADS * NJ], f32, tag="pej", name="pe")
                    for jb in range(NJ):
                        for fc in range(4):
                            nc.tensor.matmul(
                                pe[:, jb * HEADS:(jb + 1) * HEADS],
                                lhsT=lr_v[:, fc, jb * 128:(jb + 1) * 128],
                                rhs=a_v[:, fc, :],
                                start=(fc == 0), stop=(fc == 3))
                    exf = smp.tile([128, HEADS * NJ], f32, tag="exf", name="exf", bufs=3)
                    nc.scalar.activation(exf[:], pe[:], AF.Exp)
                    ex = smp.tile([128, HEADS * NJ], EXD, tag="ex", name="ex", bufs=3)
                    nc.vector.tensor_copy(ex[:], exf[:])
                    pden = pss.tile([128, HEADS], f32, tag="pej", name="pden")
                    for jb in range(NJ):
                        nc.tensor.matmul(pden[:], lhsT=sj_v[:, jb, :],
                                         rhs=ex[:, jb * HEADS:(jb + 1) * HEADS],
                                         start=(jb == 0), stop=(jb == NJ - 1))
                    rden = smp.tile([128, HEADS], f32, tag="rden", name="rden")
                    nc.vector.reciprocal(rden[:], pden[:])

                    pagg = psb.tile([128, FT], f32, tag="pbig", name="pagg")
                    for jb in range(NJ):
                        wt = sbp.tile([128, FT], EXD, tag="wt", name="wt")
                        for h in range(HEADS):
                            nc.vector.tensor_scalar_mul(
                                wt[:, h * C:(h + 1) * C],
                                xlg_v[:, jb, h * C:(h + 1) * C],
                                exf[:, jb * HEADS + h:jb * HEADS + h + 1])
                        nc.tensor.matmul(pagg[:], lhsT=sj_v[:, jb, :], rhs=wt[:],
                                         start=(jb == 0), stop=(jb == NJ - 1))
                    # epilogue: out = pagg*rden (per head) + bias
                    o1 = smp.tile([128, FT], f32, tag="o1", name="o1")
                    for h in range(HEADS):
                        nc.vector.tensor_scalar_mul(
                            o1[:, h * C:(h + 1) * C], pagg[:, h * C:(h + 1) * C],
                            rden[:, h:h + 1])
                    if layer == 1:
                        h1p = smp.tile([128, FT], f16, tag="h1p", name="h1p")
                        nc.vector.tensor_tensor(out=h1p[:], in0=o1[:],
                                                in1=b1f_s[:], op=ALU.add)
                        for fc in range(4):
                            pt = pss.tile([128, 128], f16, tag="pe", name="pt",
                                          bufs=1)
                            nc.tensor.transpose(
                                pt[:], h1p[:, fc * 128:(fc + 1) * 128],
                                ident_s[:])
                            nc.vector.tensor_copy(h1T_g[:, t, fc, :], pt[:])
                    else:
                        h2t = h2_s[:, t * FT:(t + 1) * FT]
                        nc.vector.tensor_tensor(out=h2t, in0=o1[:], in1=b2f_s[:],
                                                op=ALU.add)
                        sq = smp.tile([128, FT], f16, tag="sqs", name="sq", bufs=1)
                        nc.scalar.activation(sq[:], h2t, AF.Square,
                                             accum_out=acc16[:, t:t + 1])
                        ec = ecols_s[:, t * GPC:(t + 1) * GPC]
                        nc.tensor.matmul(pgm[:GPC, :], lhsT=ec, rhs=h2t,
                                         start=(t == 0), stop=(t == NT - 1))
                        # z0 row for graph t (node 0 of local tile t)
                        nc.sync.dma_start(z0p[t:t + 1, 1:513],
                                          h2_s[0:1, t * FT:(t + 1) * FT])
                if layer == 1:
                    # GELU over transposed h1 in 2 chunks (consecutive: one
                    # table load) so t2_xl starts on the first half early
                    H = NT * FT // 2
                    nc.scalar.activation(h1T_s[:, 0:H], h1T_s[:, 0:H], AF.Gelu)
                    nc.scalar.activation(h1T_s[:, H:2 * H], h1T_s[:, H:2 * H],
                                         AF.Gelu)
                if layer == 2:
                    # time coord for all tiles: t = sqrt(1 + acc)
                    sqrt_nr(tal[:], acc16[:], smp, "t_", x_plus=1.0)
                    nc.vector.tensor_copy(tal16[:], tal[:])
                    nc.sync.dma_start(z0p[:, 0:1], tal16[0:1, :])

            transform1()
            load_consts()
            edge_layer(1, xl1_tb, a1_s)
            t2_xl()
            nc.gpsimd.collective_compute(
                "AllGather", mybir.AluOpType.bypass, replica_groups=groups,
                ins=[xl2_sh[:]], outs=[xl2_tb[:]])
            t2_xr()
            nc.vector.memset(z0p[:], 0.0)
            nc.vector.memset(z0p[:, 513:514], 1.0)
            # mlp weights: memset-gated so the DMA lands in the AG2 window
            wa_s = pers.tile([128, 5 * 2176], f16, name="wa_s")
            nc.vector.memset(wa_s[0:1, 0:1], 0.0)
            nc.sync.dma_start(wa_s[:], wa_d[:])
            wf_s = pers.tile([128, 5 * 640], f16, name="wf_s")
            nc.vector.memset(wf_s[0:1, 0:1], 0.0)
            nc.sync.dma_start(wf_s[:], wf_d[:])
            sabf = pers.tile([16, 3], f32, name="sabf")
            nc.sync.dma_start(sabf[:], sabf_d[:])
            esc = pers.tile([16, 3], f32, name="esc")
            nc.scalar.activation(esc[:], sabf[:], AF.Exp)
            pgm = psb.tile([128, 512], f32, tag="pgm", name="pgm", bufs=1)
            edge_layer(2, xl2_tb, a2_s)

            # ---------------- graph phase: centroid ----------------
            pgm1 = pss.tile([128, 4], f32, tag="pej", name="pgm1")
            for t in range(NT):
                ec = ecols_s[:, t * GPC:(t + 1) * GPC]
                nc.tensor.matmul(pgm1[:GPC, 0:1], lhsT=ec,
                                 rhs=tal16[:, t:t + 1],
                                 start=(t == 0), stop=(t == NT - 1))
            sums = smp.tile([GPC, FT + 1], f32, tag="sums", name="sums")
            nc.vector.tensor_copy(sums[:, 1:513], pgm[:GPC, :])
            nc.vector.tensor_copy(sums[:, 0:1], pgm1[:GPC, 0:1])
            sqg = smp.tile([GPC, FT], f32, tag="sqg", name="sqg", bufs=1)
            sa_ = smp.tile([GPC, 1], f32, tag="acc", name="sa_")
            nc.scalar.activation(sqg[:], sums[:, 1:FT + 1], AF.Square,
                                 accum_out=sa_[:])
            innr = smp.tile([GPC, 1], f32, tag="in1", name="innr")
            nc.vector.tensor_tensor(out=innr[:], in0=sums[:, 0:1],
                                    in1=sums[:, 0:1], op=ALU.mult)
            nc.vector.tensor_tensor(out=innr[:], in0=innr[:], in1=sa_[:],
                                    op=ALU.subtract)
            nc.vector.tensor_scalar_max(innr[:], innr[:], 1e-8 * (N // B) ** 2)
            rt = smp.tile([GPC, 1], f32, tag="in1", name="rt")
            sqrt_nr(rt[:], innr[:], smp, "g_")
            nc.vector.reciprocal(rt[:], rt[:])
            gmt = smp.tile([GPC, FT + 1], f32, tag="sums", name="gmt")
            nc.scalar.activation(gmt[:], sums[:], AF.Copy, scale=rt[:])
            nc.sync.dma_start(gmout[:], gmt[:])

        # ---------------- Lorentz MLP on z0 [16, 513] ----------------
        with (tc.tile_pool(name="mlp", bufs=1) as mpool,
              tc.tile_pool(name="mstream", bufs=2) as msb,
              tc.tile_pool(name="mps", bufs=4, space="PSUM") as mps,
              tc.tile_pool(name="mpss", bufs=2, space="PSUM") as mpss):
            wb_s = mpool.tile([128, 17 * 640], f16, name="wb_s")
            nc.sync.dma_start(wb_s[:], wb_d[:])

            def trans_blocks(zp, kb):
                """zp [16, kb*128] f16 -> zT [128, kb*16] f16 via PE transposes."""
                zT = msb.tile([128, kb * 16], f16, tag="zT", name="zT")
                for k in range(kb):
                    pt = mpss.tile([128, 16], f16, tag="mtr", name="pt")
                    nc.tensor.transpose(pt[:], zp[:, k * 128:(k + 1) * 128],
                                        ident_s[:16, :16])
                    nc.vector.tensor_copy(zT[:, k * 16:(k + 1) * 16], pt[:])
                return zT

            def mm_thin(zT, kb, w_s, ncols):
                """out [16, ncols] f32 = zT.T @ w; w_s view [128, kb, ncols]."""
                w_v = w_s[:].rearrange("p (k n) -> p k n", k=kb)
                out = msb.tile([16, ncols], f32, tag="mlpo", name="out")
                zT_v = zT[:].rearrange("p (k n) -> p k n", k=kb)
                for o in range(0, ncols, 512):
                    n = min(512, ncols - o)
                    pm = mps.tile([128, 512], f32, tag="mbig", name="pm")
                    for k in range(kb):
                        nc.tensor.matmul(pm[:16, :n], lhsT=zT_v[:, k, :],
                                         rhs=w_v[:, k, o:o + n],
                                         start=(k == 0), stop=(k == kb - 1))
                    nc.vector.tensor_copy(out[:, o:o + n], pm[:16, :n])
                return out

            def llin_post(zz, kout, esc_idx):
                """t1 = esc*sigmoid(z0col)+1.1 (via exp), r = sqrt((t1^2-1)/sq)."""
                en = msb.tile([16, 1], f32, tag="t1", name="en", bufs=8)
                nc.scalar.activation(en[:], zz[:, 0:1], AF.Exp, scale=-1.0)
                sg = msb.tile([16, 1], f32, tag="t1", name="sg", bufs=8)
                nc.vector.tensor_scalar_add(sg[:], en[:], 1.0)
                nc.vector.reciprocal(sg[:], sg[:])
                t1 = msb.tile([16, 1], f32, tag="t1", name="t1", bufs=8)
                nc.vector.tensor_scalar(
                    out=t1[:], in0=sg[:], scalar1=esc[:, esc_idx:esc_idx + 1],
                    scalar2=1.1, op0=ALU.mult, op1=ALU.add)
                sp = zz[:, 1:kout]
                sq = msb.tile([16, kout - 1], f32, tag="msq", name="sq")
                ac = msb.tile([16, 1], f32, tag="t1", name="ac", bufs=8)
                nc.scalar.activation(sq[:], sp, AF.Square, accum_out=ac[:])
                nc.vector.tensor_scalar_max(ac[:], ac[:], 1e-8)
                r_ = msb.tile([16, 1], f32, tag="t1", name="r_", bufs=8)
                nc.vector.reciprocal(r_[:], ac[:])
                t2 = msb.tile([16, 1], f32, tag="t1", name="t2", bufs=8)
                nc.vector.tensor_tensor(out=t2[:], in0=t1[:], in1=t1[:],
                                        op=ALU.mult)
                nc.vector.tensor_scalar_add(t2[:], t2[:], -1.0)
                nc.vector.tensor_tensor(out=r_[:], in0=r_[:], in1=t2[:],
                                        op=ALU.mult)
                sqrt_nr(r_[:], r_[:], msb, "m_")
                return t1, r_

            # llin-a: z0 [16, 513] -> zA [16, 2176(junk pad)]
            zT = trans_blocks(z0p, 5)
            zA = mm_thin(zT, 5, wa_s, 2176)
            t1, r1 = llin_post(zA, 2049, 0)
            # z1 = add_time(gelu(sp*r1)): gelu with scale=r1
            z1p = msb.tile([16, 17 * 128], f16, tag="z1p", name="z1p")
            nc.vector.memset(z1p[:], 0.0)
            nc.scalar.activation(z1p[:, 1:2049], zA[:, 1:2049], AF.Gelu,
                                 scale=r1[:])
            sqz = msb.tile([16, 2048], f32, tag="msq", name="sqz")
            az = msb.tile([16, 1], f32, tag="t1", name="az", bufs=8)
            nc.scalar.activation(sqz[:], z1p[:, 1:2049], AF.Square,
                                 accum_out=az[:])
            sqrt_nr(z1p[:, 0:1], az[:], msb, "z_", x_plus=1.0)
            nc.vector.memset(z1p[:, 2049:2050], 1.0)
            # llin-b: [16, 2049] -> [16, 513]
            zTb = trans_blocks(z1p, 17)
            zB = mm_thin(zTb, 17, wb_s, 640)
            t3, r3 = llin_post(zB, 513, 1)
            z2p = msb.tile([16, 640], f16, tag="z0p", name="z2p")
            nc.vector.memset(z2p[:, 513:640], 0.0)
            nc.vector.tensor_copy(z2p[:, 0:1], t3[:])
            nc.vector.tensor_scalar_mul(z2p[:, 1:513], zB[:, 1:513], r3[:])
            nc.vector.memset(z2p[:, 513:514], 1.0)
            # llin-f: [16, 513] -> [16, 513]
            zTf = trans_blocks(z2p, 5)
            zF = mm_thin(zTf, 5, wf_s, 640)
            t4, r4 = llin_post(zF, 513, 2)
            zfin = msb.tile([16, FT + 1], f32, tag="mlpo", name="zfin")
            nc.vector.tensor_copy(zfin[:, 0:1], t4[:])
            nc.vector.tensor_scalar_mul(zfin[:, 1:513], zF[:, 1:513], r4[:])
            nc.sync.dma_start(zout[:], zfin[:])
        est.close()

    nc.compile()
    return nc


# ============================ host entry ============================

EX_DTYPE = "bfloat16"   # safe exp range


def _make_inmaps(inputs):
    x = np.asarray(inputs["x"], np.float32)
    edge_index = np.asarray(inputs["edge_index"])
    srcs, sdt, sjt, idx, LP, NJ = _prep_edges(edge_index)

    f16 = np.float16
    exd_np = ml_dtypes.bfloat16 if EX_DTYPE == "bfloat16" else np.float16

    # ---- shared (replicated) host arrays ----
    def aug5(W, b):
        return _interleave_k(_aug_w(np.asarray(W, np.float32),
                                    np.asarray(b, np.float32), 640), 5)

    w1l_h = _interleave_k(np.asarray(inputs["Wl1"], np.float32), 4
                          ).astype(f16).reshape(128, 4 * FT)
    b1r_eff = np.asarray(inputs["br1"], np.float32) + np.asarray(inputs["bl1"], np.float32)
    w1r_h = aug5(inputs["Wr1"], b1r_eff).astype(f16).reshape(128, 5 * FT)
    w2l_h = _interleave_k(np.asarray(inputs["Wl2"], np.float32), 4).astype(f16).reshape(128, 4 * FT)
    w2r_h = _interleave_k(np.asarray(inputs["Wr2"], np.float32), 4).astype(f16).reshape(128, 4 * FT)
    b2r_eff = (np.asarray(inputs["br2"], np.float32)
               + np.asarray(inputs["bl2"], np.float32))
    b2rows = b2r_eff[None, :].astype(f16)

    def amat(att):
        att = np.asarray(att, np.float32)
        A = np.zeros((FT, HEADS), np.float32)
        for hh in range(HEADS):
            A[hh * C:(hh + 1) * C, hh] = att[hh]
        return _interleave_k(A, 4).astype(f16).reshape(128, 4 * HEADS)

    a1_h, a2_h = amat(inputs["att1"]), amat(inputs["att2"])
    b1eff = (np.asarray(inputs["bias1"], np.float32)
             + np.asarray(inputs["bl1"], np.float32))
    b2eff = (np.asarray(inputs["bias2"], np.float32)
             + np.asarray(inputs["bl2"], np.float32))
    b1full = np.tile(b1eff[None, :], (128, 1))
    b2full = np.tile(b2eff[None, :], (128, 1))
    ecols = np.zeros((128, NT * GPC), f16)
    for t in range(NT):
        ecols[:, t * GPC + t] = 1.0
    ident = np.eye(128, dtype=f16)
    wa_h = _interleave_k(_aug_w(np.asarray(inputs["Wa"], np.float32),
                                np.asarray(inputs["ba"], np.float32), 640, 2176), 5
                         ).astype(f16).reshape(128, 5 * 2176)
    wb_h = _interleave_k(_aug_w(np.asarray(inputs["Wb"], np.float32),
                                np.asarray(inputs["bb"], np.float32), 17 * 128, 640), 17
                         ).astype(f16).reshape(128, 17 * 640)
    wf_h = _interleave_k(_aug_w(np.asarray(inputs["Wf"], np.float32),
                                np.asarray(inputs["bf"], np.float32), 640, 640), 5
                         ).astype(f16).reshape(128, 5 * 640)
    sabf = np.tile(np.array([[float(inputs["sa"]), float(inputs["sb"]),
                              float(inputs["sf"])]], np.float32), (16, 1))

    # full x^T table (replicated): rows 0..511 = x[:,1:].T (no bias row)
    xTf_h = _interleave_k(np.ascontiguousarray(x[:, 1:].T), 4
                          ).astype(f16).reshape(128, 4 * N)
    xTaug = np.zeros((640, N), np.float32)
    xTaug[:FT] = x[:, 1:].T
    xTaug[FT] = 1.0
    xTa5 = _interleave_k(xTaug, 5).astype(f16)

    in_maps = []
    for k in range(NCORES):
        sl = slice(k * SHARD, (k + 1) * SHARD)
        xTl_h = np.ascontiguousarray(xTa5[:, :, sl]).reshape(128, 5 * SHARD)
        tsl = slice(k * NT, (k + 1) * NT)
        in_maps.append({
            "xTf": xTf_h, "xTl": xTl_h,
            "w1l": w1l_h, "w1r": w1r_h, "w2l": w2l_h, "w2r": w2r_h,
            "b2rows": b2rows, "a1": a1_h, "a2": a2_h, "b1full": b1full,
            "b2full": b2full,
            "sdt": sdt[tsl].astype(ml_dtypes.float8_e4m3fn),
            "sj": np.ascontiguousarray(
                sjt[tsl].reshape(NT, NJ, 128, 128).transpose(0, 2, 1, 3)
            ).reshape(NT, 128, NJ * 128).astype(ml_dtypes.float8_e4m3fn),
            "idx": idx[tsl],
            "ecols": ecols, "ident": ident,
            "wa": wa_h, "wb": wb_h, "wf": wf_h, "sabf": sabf,
        })

    return in_maps, LP, NJ


_last_exec_ns = None


def kernel(**inputs):
    global _last_exec_ns
    in_maps, LP, NJ = _make_inmaps(inputs)
    key = (LP, EX_DTYPE)
    if key not in _cache:
        _cache[key] = _build(LP, NJ, EX_DTYPE)
    nc = _cache[key]
    from concourse.bass_utils import run_bass_kernel_spmd
    res = run_bass_kernel_spmd(nc, in_maps, list(range(NCORES)))
    _last_exec_ns = res.exec_time_ns
    kernel._last_res = res
    z = np.concatenate([np.asarray(r["zout"]) for r in res.results], 0)
    gm = np.concatenate([np.asarray(r["gmout"]) for r in res.results], 0)
    return z.astype(np.float32), gm.astype(np.float32)
